# revision 17
# baseline (speedup 1.0000x reference)
"""Bidirectional 2-layer RNN (B=64, T=1024, NIN=H=512) on 8 Trainium2 cores.

Sharding: 4 core-pairs x {fwd, bwd}. Pair p owns sequences [16p, 16p+16);
the even core of the pair runs the forward direction, the odd core the
backward direction (on host-time-reversed inputs, so the device program is
identical SPMD). Between layers, each core computes its own-half l1 input
projection (A) and the partner's cross-half projection (B, written
time-flipped into the partner's time frame); a pairwise ReduceScatter(add)
then hands every core its complete layer-1 pre-activations.

On-device everything lives in a transposed [hidden, time, batch] layout so
the recurrent matmuls keep the weights stationary ([128,128] bf16 tiles)
and the tanh output feeds the next step's moving operand with no transposes.
"""

import sys

sys.path.insert(0, "/opt/trn_rl_repo")

import numpy as np
import ml_dtypes

import concourse.bacc as bacc
import concourse.mybir as mybir
from concourse.tile import TileContext
from concourse.bass_utils import run_bass_kernel_spmd

BF16 = mybir.dt.bfloat16
F32 = mybir.dt.float32
ACT_TANH = mybir.ActivationFunctionType.Tanh

B, T_FULL, NIN, H = 64, 1024, 512, 512
NCORES, NPAIRS, BL = 8, 4, 16  # cores, pairs, sequences per pair
P = 128
KT, MT = H // P, H // P  # 4 k-tiles, 4 m-tiles
RG = [[0, 1], [2, 3], [4, 5], [6, 7]]  # pair replica groups

CH = 64   # scan steps per chunk
PT = 32   # proj t-steps per chunk (moving N = PT*BL = 512)


def _load_weight(nc, pool, name):
    """Host layout [512, 512] (k-major) -> SBUF [128, KT, MT, 128] bf16."""
    dram = nc.declare_dram_parameter(name, [H, H], BF16, isOutput=False)
    w = pool.tile([P, KT, MT, P], BF16, tag=name)
    for k in range(KT):
        nc.sync.dma_start(out=w[:, k, :, :], in_=dram[P * k : P * (k + 1), :])
    return w


def _load_bias(nc, pool, name):
    dram = nc.declare_dram_parameter(name, [P, MT], F32, isOutput=False)
    b = pool.tile([P, MT], F32, tag=name)
    nc.sync.dma_start(out=b[:], in_=dram[:])
    return b


def _proj(nc, tc, pools, T, src_ap, w_list, bias_list, dst_writers):
    """pre[m-tile j, t, b] = sum_k w[k,m].T @ src[k, t, b] (+bias), chunked.

    src_ap: DRAM [H(or 2H), T, BL] bf16 source (k-major).
    w_list: list of SBUF weight tiles [P, KT, MT, P] to accumulate (the
        k-tiles of consecutive list entries extend the contraction).
    dst_writers: fn(ch, t0, stage_tile) emitting the DMA-out(s) for the
        f32 stage tile [P, (MT, PT, BL)].
    """
    mv_pool, ps_pool, st_pool = pools
    nk = src_ap.shape[0] // H  # 1 for l0, ...; w_list entries are per-512 block
    for ch in range(T // PT):
        t0 = ch * PT
        mv = mv_pool.tile([P, nk * KT, PT, BL], BF16, tag="proj_mv")
        for kb in range(nk * KT):
            nc.sync.dma_start(
                out=mv[:, kb, :, :],
                in_=src_ap[P * kb : P * (kb + 1), t0 : t0 + PT, :],
            )
        stage = st_pool.tile([P, MT, PT, BL], F32, tag="proj_stage")
        for m in range(MT):
            ps = ps_pool.tile([P, PT * BL], F32, tag="proj_ps")
            first = True
            for blk, w in enumerate(w_list):
                for k in range(KT):
                    nc.tensor.matmul(
                        ps[:],
                        w[:, k, m, :],
                        mv[:, blk * KT + k, :, :],
                        start=first,
                        stop=(blk == len(w_list) - 1 and k == KT - 1),
                    )
                    first = False
            if bias_list is not None:
                nc.vector.tensor_scalar_add(
                    stage[:, m, :, :], ps[:], bias_list[:, m : m + 1]
                )
            else:
                nc.vector.tensor_copy(stage[:, m, :, :], ps[:])
        dst_writers(ch, t0, stage)


def _scan(nc, tc, pools, T, whh, pre_ap, out_writer):
    """Recurrent tanh scan in transposed layout.

    pre_ap: DRAM [H, T, BL] f32 pre-activations (m-major j tiles).
    out_writer: fn(ch, t0, stag_tile) -> emit DMA-out for the bf16 staging
        tile [P, (KT, CH, BL)] holding h.T for steps [t0, t0+CH).
    """
    pre_pool, stag_pool, ps_pool = pools
    prev_stag = None
    for ch in range(T // CH):
        t0 = ch * CH
        pre_t = pre_pool.tile([P, MT, CH, BL], F32, tag="scan_pre")
        for m in range(MT):
            nc.sync.dma_start(
                out=pre_t[:, m, :, :],
                in_=pre_ap[P * m : P * (m + 1), t0 : t0 + CH, :],
            )
        stag = stag_pool.tile([P, KT, CH, BL], BF16, tag="scan_stag")
        for t in range(CH):
            s = t0 + t
            if s == 0:
                nc.scalar.activation(
                    stag[:, :, 0, :], pre_t[:, :, 0, :], ACT_TANH
                )
                continue
            hp = stag[:, :, t - 1, :] if t > 0 else prev_stag[:, :, CH - 1, :]
            ps = ps_pool.tile([P, MT * BL], F32, tag="scan_ps")
            for m in range(MT):
                for k in range(KT):
                    nc.tensor.matmul(
                        ps[:, m * BL : (m + 1) * BL],
                        whh[:, k, m, :],
                        hp[:, k, :],
                        start=(k == 0),
                        stop=(k == KT - 1),
                    )
            nc.vector.tensor_add(ps[:], ps[:], pre_t[:, :, t, :])
            nc.scalar.activation(stag[:, :, t, :], ps[:], ACT_TANH)
        out_writer(ch, t0, stag)
        prev_stag = stag


def build_nc(T, no_cc=False, no_cond=False, no_negstride=False, dbg=False):
    nc = bacc.Bacc(num_devices=NCORES)

    xT = nc.declare_dram_parameter("xT", [NIN, T, BL], BF16, isOutput=False)
    out1T = nc.declare_dram_parameter("out1T", [H, T, BL], BF16, isOutput=True)
    sel = nc.declare_dram_parameter("sel", [1, 2], mybir.dt.uint32, isOutput=False)

    pre0T = nc.dram_tensor("pre0T", [H, T, BL], F32)
    out0T = nc.dram_tensor("out0T", [H, T, BL], BF16)
    u_rs = nc.dram_tensor("u_rs", [2, H, T, BL], F32)
    pre1T = nc.dram_tensor("pre1T", [H, T, BL], F32)

    with TileContext(nc) as tc:
        with tc.tile_pool(name="const", bufs=1) as cpool:
            wih0 = _load_weight(nc, cpool, "wih0T")
            whh0 = _load_weight(nc, cpool, "whh0T")
            wih1o = _load_weight(nc, cpool, "wih1ownT")
            wcross = _load_weight(nc, cpool, "wcrossT")
            whh1 = _load_weight(nc, cpool, "whh1T")
            bias0 = _load_bias(nc, cpool, "bias0")
            bias1 = _load_bias(nc, cpool, "bias1")
            sel_sb = cpool.tile([1, 2], mybir.dt.uint32, tag="sel")
            nc.sync.dma_start(out=sel_sb[:], in_=sel[:])
            if no_cond:
                va = vb = None
            else:
                va = nc.values_load(
                    sel_sb[0:1, 0:1], min_val=0, max_val=1,
                    skip_runtime_bounds_check=True,
                )
                vb = nc.values_load(
                    sel_sb[0:1, 1:2], min_val=0, max_val=1,
                    skip_runtime_bounds_check=True,
                )

            with tc.tile_pool(name="mv", bufs=3) as mvp, \
                 tc.tile_pool(name="pps", bufs=4, space="PSUM") as ppsp, \
                 tc.tile_pool(name="pst", bufs=2) as pstp:

                def w_pre0(ch, t0, stage):
                    for m in range(MT):
                        nc.sync.dma_start(
                            out=pre0T[P * m : P * (m + 1), t0 : t0 + PT, :],
                            in_=stage[:, m, :, :],
                        )

                _proj(nc, tc, (mvp, ppsp, pstp), T, xT[:], [wih0], bias0, w_pre0)

            with tc.tile_pool(name="spre", bufs=3) as sprep, \
                 tc.tile_pool(name="sstag", bufs=3) as sstagp, \
                 tc.tile_pool(name="sps", bufs=2, space="PSUM") as spsp:

                def w_out0(ch, t0, stag):
                    for k in range(KT):
                        nc.sync.dma_start(
                            out=out0T[P * k : P * (k + 1), t0 : t0 + CH, :],
                            in_=stag[:, k, :, :],
                        )

                _scan(nc, tc, (sprep, sstagp, spsp), T, whh0, pre0T[:], w_out0)

            # layer-1 projections: A (own half, +bias1) and B (cross half,
            # written time-flipped into the partner's slot).
            with tc.tile_pool(name="mv1", bufs=3) as mvp, \
                 tc.tile_pool(name="pps1", bufs=8, space="PSUM") as ppsp, \
                 tc.tile_pool(name="pst1", bufs=4) as pstp:

                def w_a(ch, t0, stage):
                    for m in range(MT):
                        for s, cond in ((0, va), (1, vb)):
                            if no_cond:
                                if s != 0:
                                    continue
                                cond = None
                            nc.sync.dma_start(
                                out=u_rs[s, P * m : P * (m + 1), t0 : t0 + PT, :],
                                in_=stage[:, m, :, :],
                                cond=cond,
                            )

                _proj(nc, tc, (mvp, ppsp, pstp), T, out0T[:], [wih1o], bias1, w_a)

                def w_b(ch, t0, stage):
                    # time-flip into the partner's frame: stage step tt lands
                    # at partner-time T-1-(t0+tt). The flip is done with a
                    # reversed SBUF-side read (a negative stride on the DRAM
                    # side breaks the cond/skip bounds check and silently
                    # drops the transfer).
                    tsl = slice(T - PT - t0, T - t0)
                    for m in range(MT):
                        for s, cond in ((1, va), (0, vb)):
                            if no_cond:
                                if s != 1:
                                    continue
                                cond = None
                            nc.sync.dma_start(
                                out=u_rs[s, P * m : P * (m + 1), tsl, :],
                                in_=stage[:, m, ::-1, :]
                                if not no_negstride
                                else stage[:, m, :, :],
                                cond=cond,
                            )

                _proj(nc, tc, (mvp, ppsp, pstp), T, out0T[:], [wcross], None, w_b)

            tc.strict_bb_all_engine_barrier()
            if no_cc:
                for m in range(MT):
                    nc.sync.dma_start(
                        out=pre1T[P * m : P * (m + 1), :, :],
                        in_=u_rs[0, P * m : P * (m + 1), :, :],
                    )
            else:
                nc.gpsimd.collective_compute(
                    "ReduceScatter",
                    mybir.AluOpType.add,
                    replica_groups=RG,
                    ins=[u_rs[:].rearrange("s h t b -> (s h t b)")],
                    outs=[pre1T[:].rearrange("h t b -> (h t b)")],
                )
            tc.strict_bb_all_engine_barrier()

            if dbg:
                u_dbg = nc.declare_dram_parameter(
                    "u_dbg", [2, H, T, BL], F32, isOutput=True
                )
                pre1_dbg = nc.declare_dram_parameter(
                    "pre1_dbg", [H, T, BL], F32, isOutput=True
                )
                nc.sync.dma_start(out=u_dbg[:], in_=u_rs[:])
                nc.sync.dma_start(out=pre1_dbg[:], in_=pre1T[:])
                tc.strict_bb_all_engine_barrier()

            with tc.tile_pool(name="spre1", bufs=3) as sprep, \
                 tc.tile_pool(name="sstag1", bufs=3) as sstagp, \
                 tc.tile_pool(name="sps1", bufs=2, space="PSUM") as spsp:

                def w_out1(ch, t0, stag):
                    for k in range(KT):
                        nc.sync.dma_start(
                            out=out1T[P * k : P * (k + 1), t0 : t0 + CH, :],
                            in_=stag[:, k, :, :],
                        )

                _scan(nc, tc, (sprep, sstagp, spsp), T, whh1, pre1T[:], w_out1)

    if not nc.is_finalized():
        nc.finalize()
    return nc


def _bf16(a):
    return np.ascontiguousarray(a).astype(ml_dtypes.bfloat16)


def make_in_maps(inputs, T):
    x = np.asarray(inputs["input_feat"])  # [B, T, NIN] f32
    maps = []
    for p in range(NPAIRS):
        seqs = slice(BL * p, BL * (p + 1))
        for par, d in ((0, "f"), (1, "b")):
            dp = "b" if d == "f" else "f"
            xs = x[seqs, :T]
            if par == 1:
                xs = xs[:, ::-1]
            col = slice(0, H) if par == 0 else slice(H, 2 * H)
            w1o = np.asarray(inputs[f"w_ih_1{d}"])[:, col]
            w1c = np.asarray(inputs[f"w_ih_1{dp}"])[:, col]
            m = {
                "xT": _bf16(xs.transpose(2, 1, 0)),
                "wih0T": _bf16(np.asarray(inputs[f"w_ih_0{d}"]).T),
                "whh0T": _bf16(np.asarray(inputs[f"w_hh_0{d}"]).T),
                "wih1ownT": _bf16(w1o.T),
                "wcrossT": _bf16(w1c.T),
                "whh1T": _bf16(np.asarray(inputs[f"w_hh_1{d}"]).T),
                "bias0": np.ascontiguousarray(
                    (np.asarray(inputs[f"b_ih_0{d}"]) + np.asarray(inputs[f"b_hh_0{d}"]))
                    .reshape(MT, P).T.astype(np.float32)
                ),
                "bias1": np.ascontiguousarray(
                    (np.asarray(inputs[f"b_ih_1{d}"]) + np.asarray(inputs[f"b_hh_1{d}"]))
                    .reshape(MT, P).T.astype(np.float32)
                ),
                "sel": np.array([[1 - par, par]], dtype=np.uint32),
            }
            maps.append(m)
    return maps


def assemble_output(results, T):
    y = np.empty((B, T, 2 * H), dtype=np.float32)
    for p in range(NPAIRS):
        seqs = slice(BL * p, BL * (p + 1))
        for par in (0, 1):
            o = np.asarray(results[2 * p + par]["out1T"]).astype(np.float32)
            o = o.transpose(2, 1, 0)  # [BL, T, H]
            if par == 1:
                o = o[:, ::-1]
            y[seqs, :, par * H : (par + 1) * H] = o
    return y


def run(inputs, T=T_FULL, trace=False, trace_cores=None):
    nc = build_nc(T)
    in_maps = make_in_maps(inputs, T)
    res = run_bass_kernel_spmd(
        nc, in_maps, list(range(NCORES)), trace=trace, trace_cores=trace_cores
    )
    return assemble_output(res.results, T), res


def kernel(**inputs):
    out, _ = run(inputs, T=T_FULL, trace=False)
    return out
